# revision 41
# baseline (speedup 1.0000x reference)
"""DenseEnergyLoss Trainium2 kernel — separable-kernel formulation.

loss = WEIGHT * (-1/n) * sum_k A_k^T G B_k   (per image, P = 64*64 pixels)

The bilateral kernel factorizes: G[i,j] = Gs[i,j] * Gc[i,j] with
  Gs[i,j] = exp(-((xi-xj)^2+(yi-yj)^2)/(2*50^2)) = gy[yi,yj]*gx[xi,xj]
  Gc[i,j] = exp(-|ci-cj|^2/2) = ec_i * ec_j * exp(ci.cj),  c = rgb/15.
Gs is exactly separable (two 64x64 matrices). |ci.cj| <~ 0.2, so
exp(ci.cj) ~= 1 + ci.cj = sum_r phi_r(i) psi_r(j), rank R=4 (rel err ~3e-5).
Folding ec and the rank factors into A/B gives extended classes m=(k,r),
M = 21*4 = 84 per image:

  E = sum_m <(gy (x) I) Abar_m, (I (x) gx) Bbar_m>

Device (2 cores per image, Mc=42 extended classes each, 21 pairs):
  T-stage: per class-pair j, matmul with lhsT = Bbar pair slab [64x, 128(m2,y)]
    and rhs = gx [64,64]: out[(m2,y), x'] — contracting x flips y into the
    output partition, so the result lands directly in the A-side layout.
  P-stage: one matmul per chunk: lhsT = blockdiag(gy,gy) [128,128], rhs = Ay
    [128(m2,y), chunk*64] — applies gy to the packed A side.
  Reduce: fused multiply+reduce (DVE tensor_tensor_reduce / GpSimd
    scalar_tensor_tensor) of the two PSUM chunks into acc columns.
Host sums the per-core [128, n_chunk] partials.
"""

import numpy as np
import ml_dtypes

WEIGHT = 1e-07
SIGMA_RGB = 15.0
SIGMA_XY_EFF = 50.0  # SIGMA_XY * SCALE
IGNORE_LABEL = 255

N_IMG = 4
K_CLS = 21
H_DS = 64
R_COLOR = 4  # 1 + rgb linear Taylor ranks
M_EXT = K_CLS * R_COLOR  # 84
MC = M_EXT // 2  # 42 per core
N_PAIRS = MC // 2  # 21
CHUNKS = [(0, 8), (8, 8), (16, 5)]  # (first pair, n pairs)
NCH = len(CHUNKS)
NB_HALF0 = (N_PAIRS + 1) // 2  # 11 column blocks in row-half 0

BF16 = ml_dtypes.bfloat16

_CACHE = {}


def _build_program():
    import concourse.bacc as bacc
    import concourse.tile as tile
    from concourse import mybir

    f32 = mybir.dt.float32
    bf16 = mybir.dt.bfloat16

    nc = bacc.Bacc("TRN2", target_bir_lowering=False, debug=False)

    # bxt DRAM pieces: [gxr | pairs 0-4], [pairs 5-12], [pairs 13-20]
    bxg_d = nc.dram_tensor("bxg", [64, 64 + 5 * 128], bf16, kind="ExternalInput")
    bx1_d = nc.dram_tensor("bx1", [64, 8 * 128], bf16, kind="ExternalInput")
    bx2_d = nc.dram_tensor("bx2", [64, 8 * 128], bf16, kind="ExternalInput")
    # ayg DRAM pieces ([gyd | ay] layout): [gyd + ay chunk0], [ay chunks 1-2]
    aya_d = nc.dram_tensor("aya", [128, 128 + 8 * 64], bf16, kind="ExternalInput")
    ayb_d = nc.dram_tensor(
        "ayb", [128, (N_PAIRS - 8) * 64], bf16, kind="ExternalInput"
    )
    acc_d = nc.dram_tensor("acc_out", [128, NCH], f32, kind="ExternalOutput")

    with tile.TileContext(nc) as tc:
        with (
            tc.tile_pool(name="const", bufs=1) as cpool,
            tc.tile_pool(name="tpsum", bufs=1, space="PSUM") as tpool,
            tc.tile_pool(name="ppsum", bufs=1, space="PSUM") as ppool,
            tc.tile_pool(name="scr", bufs=1) as scrpool,
        ):
            # bxt: [gxr | pair blocks 0..20]; ayg: [gyd | ay]
            bxt = cpool.tile([64, 64 + N_PAIRS * 128], bf16, tag="bxt")
            ayg = cpool.tile([128, 128 + N_PAIRS * 64], bf16, tag="ayg")
            acc = cpool.tile([128, NCH], f32, tag="acc")
            B1 = 64 + 5 * 128  # bxt DMA split points
            B2 = 64 + 13 * 128

            nc.scalar.dma_start(ayg[:, : 128 + 512], aya_d[:])
            nc.sync.dma_start(bxt[:, :B1], bxg_d[:])
            nc.scalar.dma_start(bxt[:, B1:B2], bx1_d[:])
            nc.sync.dma_start(ayg[:, 128 + 512 :], ayb_d[:])
            nc.sync.dma_start(bxt[:, B2:], bx2_d[:])

            for c, (p0, npair) in enumerate(CHUNKS):
                fs = npair * 64
                tp = tpool.tile([128, fs], f32, tag=f"t{c}")
                pp = ppool.tile([128, fs], f32, tag=f"p{c}")
                for jj in range(npair):
                    j = p0 + jj
                    nc.tensor.matmul(
                        tp[:, jj * 64 : (jj + 1) * 64],
                        bxt[:, 64 + 128 * j : 192 + 128 * j],
                        bxt[:, 0:64],
                        start=True,
                        stop=True,
                        tile_position=(0, 0),
                        skip_group_check=True,
                    )
                nc.tensor.matmul(
                    pp[:],
                    ayg[:, 0:128],
                    ayg[:, 128 + p0 * 64 : 128 + (p0 + npair) * 64],
                    start=True,
                    stop=True,
                    tile_position=(0, 0),
                )
                tsb = scrpool.tile([128, fs], f32, tag=f"tsb{c}")
                nc.scalar.activation(
                    tsb[:], tp[:], mybir.ActivationFunctionType.Copy
                )
                scr = scrpool.tile([128, fs], f32, tag=f"s{c}")
                nc.vector.tensor_tensor(
                    scr[:], pp[:], tsb[:], mybir.AluOpType.mult
                )
                if c == 1:
                    scrb = scrpool.tile([128, fs], bf16, tag=f"sb{c}")
                    nc.scalar.activation(
                        scrb[:],
                        scr[:],
                        mybir.ActivationFunctionType.Copy,
                        accum_out=acc[:, c : c + 1],
                    )
                else:
                    nc.vector.reduce_sum(
                        acc[:, c : c + 1], scr[:], axis=mybir.AxisListType.X
                    )

            nc.sync.dma_start(acc_d[:], acc[:])

    nc.compile()
    return nc


def _host_prep(images, segmentations, ROIs, seg_label):
    """Returns the 8 per-core input dicts. Core c -> image c//2, half c%2."""
    imgs = images[:, :, ::2, ::2].astype(np.float64)  # [N,3,64,64]
    segs = (
        segmentations.astype(np.float64)
        .reshape(N_IMG, K_CLS, H_DS, 2, H_DS, 2)
        .mean(axis=(3, 5))
    )  # [N,21,64,64]
    rois = ROIs[:, ::2, ::2].astype(np.float64)
    lbl = seg_label[:, 0, ::2, ::2]
    unlabel = lbl == IGNORE_LABEL

    seg_max = segs.max(axis=1)
    gate = rois - seg_max
    gate = np.where(unlabel, 1.0, gate)
    gate = np.maximum(gate, 0.0)  # [N,64,64]
    seg_r = segs * rois[:, None]  # [N,21,64,64]

    c = imgs / SIGMA_RGB  # [N,3,64,64]
    ec = np.exp(-0.5 * (c**2).sum(axis=1))  # [N,64,64]
    A = seg_r * (gate * ec)[:, None]  # [N,21,64,64]
    B = seg_r * ec[:, None]

    # extended classes m = k*4 + r: r=0 -> (1,1); r=1..3 -> (c_a, c_a)
    phi = np.concatenate([np.ones((N_IMG, 1, H_DS, H_DS)), c], axis=1)  # [N,4,64,64]
    Abar = (A[:, :, None] * phi[:, None]).reshape(N_IMG, M_EXT, H_DS, H_DS)
    Bbar = (B[:, :, None] * phi[:, None]).reshape(N_IMG, M_EXT, H_DS, H_DS)

    idx = np.arange(H_DS, dtype=np.float64)
    g1 = np.exp(-0.5 * ((idx[:, None] - idx[None, :]) / SIGMA_XY_EFF) ** 2)
    g1_16 = g1.astype(BF16)
    gxr = g1_16  # [64,64]
    gyd = np.zeros((128, 128), BF16)
    gyd[0:64, 0:64] = g1_16
    gyd[64:128, 64:128] = g1_16

    in_maps = []
    for core in range(8):
        img_i = core // 2
        half = core % 2
        Al = Abar[img_i, half * MC : (half + 1) * MC]  # [42,64,64]
        Bl = Bbar[img_i, half * MC : (half + 1) * MC]

        # ay: [(m2,y), (pair, x)]; gyd prepended
        A4 = Al.reshape(N_PAIRS, 2, H_DS, H_DS)  # [j, m2, y, x]
        ayf = (
            np.ascontiguousarray(A4.transpose(1, 2, 0, 3))
            .reshape(128, N_PAIRS * 64)
            .astype(BF16)
        )
        aya = np.concatenate([gyd, ayf[:, :512]], axis=1)
        ayb = np.ascontiguousarray(ayf[:, 512:])

        # bx[x, 128j + m2*64 + y] = Bl[2j+m2, y, x]; gxr prepended
        B4 = Bl.reshape(N_PAIRS, 2, H_DS, H_DS)  # [j, m2, y, x]
        bxf = (
            np.ascontiguousarray(B4.transpose(3, 0, 1, 2))
            .reshape(64, N_PAIRS * 128)
            .astype(BF16)
        )
        bxg = np.concatenate([gxr, bxf[:, : 5 * 128]], axis=1)
        bx1 = np.ascontiguousarray(bxf[:, 5 * 128 : 13 * 128])
        bx2 = np.ascontiguousarray(bxf[:, 13 * 128 :])

        in_maps.append(
            {"bxg": bxg, "bx1": bx1, "bx2": bx2, "aya": aya, "ayb": ayb}
        )
    return in_maps


def _get_program():
    if "nc" not in _CACHE:
        _CACHE["nc"] = _build_program()
    return _CACHE["nc"]


def _install_profile_hook():
    """Best-effort registration of the axon NTFF profile hook so that
    trace=True works (used by test harness, not the plain kernel path)."""
    import sys
    import types

    if "antenv.axon_hooks" in sys.modules:
        return
    try:
        from trn_agent_boot.trn_boot import _ntff_profile_via_ctypes

        hook = _ntff_profile_via_ctypes("/opt/axon/libaxon_pjrt.so")
        mod = types.ModuleType("antenv.axon_hooks")
        mod.get_axon_ntff_profile_hook = lambda: hook
        sys.modules["antenv.axon_hooks"] = mod
    except Exception:
        pass


def kernel(images, segmentations, ROIs, seg_label, _trace=False, _tmpdir=None):
    from concourse import bass_utils

    in_maps = _host_prep(images, segmentations, ROIs, seg_label)
    nc = _get_program()
    if _trace:
        _install_profile_hook()
        bass_utils.upload_artifacts = lambda tmpdir: f"local:{tmpdir}"
    res = bass_utils.run_bass_kernel_spmd(
        nc, in_maps, list(range(8)), trace=_trace, tmpdir=_tmpdir
    )
    total = 0.0
    for r in res.results:
        total += r["acc_out"].astype(np.float64).sum()
    loss = np.float32(-WEIGHT / N_IMG * total)
    if _trace:
        return np.array([loss], np.float32), res
    return np.array([loss], np.float32)


# revision 42
# speedup vs baseline: 1.0357x; 1.0357x over previous
"""DenseEnergyLoss Trainium2 kernel — separable-kernel formulation.

loss = WEIGHT * (-1/n) * sum_k A_k^T G B_k   (per image, P = 64*64 pixels)

The bilateral kernel factorizes: G[i,j] = Gs[i,j] * Gc[i,j] with
  Gs[i,j] = exp(-((xi-xj)^2+(yi-yj)^2)/(2*50^2)) = gy[yi,yj]*gx[xi,xj]
  Gc[i,j] = exp(-|ci-cj|^2/2) = ec_i * ec_j * exp(ci.cj),  c = rgb/15.
Gs is exactly separable (two 64x64 matrices). |ci.cj| <~ 0.2, so
exp(ci.cj) ~= 1 + ci.cj = sum_r phi_r(i) psi_r(j), rank R=4 (rel err ~3e-5).
Folding ec and the rank factors into A/B gives extended classes m=(k,r),
M = 21*4 = 84 per image:

  E = sum_m <(gy (x) I) Abar_m, (I (x) gx) Bbar_m>

Device (2 cores per image, Mc=42 extended classes each, 21 pairs):
  T-stage: per class-pair j, matmul with lhsT = Bbar pair slab [64x, 128(m2,y)]
    and rhs = gx [64,64]: out[(m2,y), x'] — contracting x flips y into the
    output partition, so the result lands directly in the A-side layout.
  P-stage: one matmul per chunk: lhsT = blockdiag(gy,gy) [128,128], rhs = Ay
    [128(m2,y), chunk*64] — applies gy to the packed A side.
  Reduce: fused multiply+reduce (DVE tensor_tensor_reduce / GpSimd
    scalar_tensor_tensor) of the two PSUM chunks into acc columns.
Host sums the per-core [128, n_chunk] partials.
"""

import numpy as np
import ml_dtypes

WEIGHT = 1e-07
SIGMA_RGB = 15.0
SIGMA_XY_EFF = 50.0  # SIGMA_XY * SCALE
IGNORE_LABEL = 255

N_IMG = 4
K_CLS = 21
H_DS = 64
R_COLOR = 4  # 1 + rgb linear Taylor ranks
M_EXT = K_CLS * R_COLOR  # 84
MC = M_EXT // 2  # 42 per core
N_PAIRS = MC // 2  # 21
CHUNKS = [(0, 8), (8, 8), (16, 5)]  # (first pair, n pairs)
NCH = len(CHUNKS)
NB_HALF0 = (N_PAIRS + 1) // 2  # 11 column blocks in row-half 0

BF16 = ml_dtypes.bfloat16

_CACHE = {}


def _build_program():
    import concourse.bacc as bacc
    import concourse.tile as tile
    from concourse import mybir

    f32 = mybir.dt.float32
    bf16 = mybir.dt.bfloat16

    nc = bacc.Bacc("TRN2", target_bir_lowering=False, debug=False)

    # bxt DRAM pieces: [gxr | pairs 0-4], [pairs 5-12], [pairs 13-20]
    bxg_d = nc.dram_tensor("bxg", [64, 64 + 5 * 128], bf16, kind="ExternalInput")
    bx1_d = nc.dram_tensor("bx1", [64, 8 * 128], bf16, kind="ExternalInput")
    bx2_d = nc.dram_tensor("bx2", [64, 8 * 128], bf16, kind="ExternalInput")
    # ayg DRAM pieces ([gyd | ay] layout): [gyd + ay chunk0], [ay chunks 1-2]
    aya_d = nc.dram_tensor("aya", [128, 128 + 8 * 64], bf16, kind="ExternalInput")
    ayb_d = nc.dram_tensor(
        "ayb", [128, (N_PAIRS - 8) * 64], bf16, kind="ExternalInput"
    )
    acc_d = nc.dram_tensor("acc_out", [128, NCH], f32, kind="ExternalOutput")

    with tile.TileContext(nc) as tc:
        with (
            tc.tile_pool(name="const", bufs=1) as cpool,
            tc.tile_pool(name="tpsum", bufs=1, space="PSUM") as tpool,
            tc.tile_pool(name="ppsum", bufs=1, space="PSUM") as ppool,
            tc.tile_pool(name="scr", bufs=1) as scrpool,
        ):
            # bxt: [gxr | pair blocks 0..20]; ayg: [gyd | ay]
            bxt = cpool.tile([64, 64 + N_PAIRS * 128], bf16, tag="bxt")
            ayg = cpool.tile([128, 128 + N_PAIRS * 64], bf16, tag="ayg")
            acc = cpool.tile([128, NCH], f32, tag="acc")
            B1 = 64 + 5 * 128  # bxt DMA split points
            B2 = 64 + 13 * 128

            nc.scalar.dma_start(ayg[:, : 128 + 512], aya_d[:])
            nc.sync.dma_start(bxt[:, :B1], bxg_d[:])
            nc.scalar.dma_start(bxt[:, B1:B2], bx1_d[:])
            nc.sync.dma_start(ayg[:, 128 + 512 :], ayb_d[:])
            nc.sync.dma_start(bxt[:, B2:], bx2_d[:])

            for c, (p0, npair) in enumerate(CHUNKS):
                fs = npair * 64
                tp = tpool.tile([128, fs], f32, tag=f"t{c}")
                pp = ppool.tile([128, fs], f32, tag=f"p{c}")
                for jj in range(npair):
                    j = p0 + jj
                    nc.tensor.matmul(
                        tp[:, jj * 64 : (jj + 1) * 64],
                        bxt[:, 64 + 128 * j : 192 + 128 * j],
                        bxt[:, 0:64],
                        start=True,
                        stop=True,
                        tile_position=(0, 0),
                        skip_group_check=True,
                    )
                nc.tensor.matmul(
                    pp[:],
                    ayg[:, 0:128],
                    ayg[:, 128 + p0 * 64 : 128 + (p0 + npair) * 64],
                    start=True,
                    stop=True,
                    tile_position=(0, 0),
                )
                tsb = scrpool.tile([128, fs], f32, tag=f"tsb{c}")
                nc.scalar.activation(
                    tsb[:], tp[:], mybir.ActivationFunctionType.Copy
                )
                scr = scrpool.tile([128, fs], f32, tag=f"s{c}")
                nc.vector.scalar_tensor_tensor(
                    out=scr[:],
                    in0=pp[:],
                    scalar=1.0,
                    in1=tsb[:],
                    op0=mybir.AluOpType.mult,
                    op1=mybir.AluOpType.mult,
                    accum_out=acc[:, c : c + 1],
                )

            nc.sync.dma_start(acc_d[:], acc[:])

    nc.compile()
    return nc


def _host_prep(images, segmentations, ROIs, seg_label):
    """Returns the 8 per-core input dicts. Core c -> image c//2, half c%2."""
    imgs = images[:, :, ::2, ::2].astype(np.float64)  # [N,3,64,64]
    segs = (
        segmentations.astype(np.float64)
        .reshape(N_IMG, K_CLS, H_DS, 2, H_DS, 2)
        .mean(axis=(3, 5))
    )  # [N,21,64,64]
    rois = ROIs[:, ::2, ::2].astype(np.float64)
    lbl = seg_label[:, 0, ::2, ::2]
    unlabel = lbl == IGNORE_LABEL

    seg_max = segs.max(axis=1)
    gate = rois - seg_max
    gate = np.where(unlabel, 1.0, gate)
    gate = np.maximum(gate, 0.0)  # [N,64,64]
    seg_r = segs * rois[:, None]  # [N,21,64,64]

    c = imgs / SIGMA_RGB  # [N,3,64,64]
    ec = np.exp(-0.5 * (c**2).sum(axis=1))  # [N,64,64]
    A = seg_r * (gate * ec)[:, None]  # [N,21,64,64]
    B = seg_r * ec[:, None]

    # extended classes m = k*4 + r: r=0 -> (1,1); r=1..3 -> (c_a, c_a)
    phi = np.concatenate([np.ones((N_IMG, 1, H_DS, H_DS)), c], axis=1)  # [N,4,64,64]
    Abar = (A[:, :, None] * phi[:, None]).reshape(N_IMG, M_EXT, H_DS, H_DS)
    Bbar = (B[:, :, None] * phi[:, None]).reshape(N_IMG, M_EXT, H_DS, H_DS)

    idx = np.arange(H_DS, dtype=np.float64)
    g1 = np.exp(-0.5 * ((idx[:, None] - idx[None, :]) / SIGMA_XY_EFF) ** 2)
    g1_16 = g1.astype(BF16)
    gxr = g1_16  # [64,64]
    gyd = np.zeros((128, 128), BF16)
    gyd[0:64, 0:64] = g1_16
    gyd[64:128, 64:128] = g1_16

    in_maps = []
    for core in range(8):
        img_i = core // 2
        half = core % 2
        Al = Abar[img_i, half * MC : (half + 1) * MC]  # [42,64,64]
        Bl = Bbar[img_i, half * MC : (half + 1) * MC]

        # ay: [(m2,y), (pair, x)]; gyd prepended
        A4 = Al.reshape(N_PAIRS, 2, H_DS, H_DS)  # [j, m2, y, x]
        ayf = (
            np.ascontiguousarray(A4.transpose(1, 2, 0, 3))
            .reshape(128, N_PAIRS * 64)
            .astype(BF16)
        )
        aya = np.concatenate([gyd, ayf[:, :512]], axis=1)
        ayb = np.ascontiguousarray(ayf[:, 512:])

        # bx[x, 128j + m2*64 + y] = Bl[2j+m2, y, x]; gxr prepended
        B4 = Bl.reshape(N_PAIRS, 2, H_DS, H_DS)  # [j, m2, y, x]
        bxf = (
            np.ascontiguousarray(B4.transpose(3, 0, 1, 2))
            .reshape(64, N_PAIRS * 128)
            .astype(BF16)
        )
        bxg = np.concatenate([gxr, bxf[:, : 5 * 128]], axis=1)
        bx1 = np.ascontiguousarray(bxf[:, 5 * 128 : 13 * 128])
        bx2 = np.ascontiguousarray(bxf[:, 13 * 128 :])

        in_maps.append(
            {"bxg": bxg, "bx1": bx1, "bx2": bx2, "aya": aya, "ayb": ayb}
        )
    return in_maps


def _get_program():
    if "nc" not in _CACHE:
        _CACHE["nc"] = _build_program()
    return _CACHE["nc"]


def _install_profile_hook():
    """Best-effort registration of the axon NTFF profile hook so that
    trace=True works (used by test harness, not the plain kernel path)."""
    import sys
    import types

    if "antenv.axon_hooks" in sys.modules:
        return
    try:
        from trn_agent_boot.trn_boot import _ntff_profile_via_ctypes

        hook = _ntff_profile_via_ctypes("/opt/axon/libaxon_pjrt.so")
        mod = types.ModuleType("antenv.axon_hooks")
        mod.get_axon_ntff_profile_hook = lambda: hook
        sys.modules["antenv.axon_hooks"] = mod
    except Exception:
        pass


def kernel(images, segmentations, ROIs, seg_label, _trace=False, _tmpdir=None):
    from concourse import bass_utils

    in_maps = _host_prep(images, segmentations, ROIs, seg_label)
    nc = _get_program()
    if _trace:
        _install_profile_hook()
        bass_utils.upload_artifacts = lambda tmpdir: f"local:{tmpdir}"
    res = bass_utils.run_bass_kernel_spmd(
        nc, in_maps, list(range(8)), trace=_trace, tmpdir=_tmpdir
    )
    total = 0.0
    for r in res.results:
        total += r["acc_out"].astype(np.float64).sum()
    loss = np.float32(-WEIGHT / N_IMG * total)
    if _trace:
        return np.array([loss], np.float32), res
    return np.array([loss], np.float32)


# revision 48
# speedup vs baseline: 1.0906x; 1.0529x over previous
"""DenseEnergyLoss Trainium2 kernel — separable-kernel formulation.

loss = WEIGHT * (-1/n) * sum_k A_k^T G B_k   (per image, P = 64*64 pixels)

The bilateral kernel factorizes: G[i,j] = Gs[i,j] * Gc[i,j] with
  Gs[i,j] = exp(-((xi-xj)^2+(yi-yj)^2)/(2*50^2)) = gy[yi,yj]*gx[xi,xj]
  Gc[i,j] = exp(-|ci-cj|^2/2) = ec_i * ec_j * exp(ci.cj),  c = rgb/15.
Gs is exactly separable (two 64x64 matrices). |ci.cj| <~ 0.2, so
exp(ci.cj) ~= 1 + ci.cj = sum_r phi_r(i) psi_r(j), rank R=4 (rel err ~3e-5).
Folding ec and the rank factors into A/B gives extended classes m=(k,r),
M = 21*4 = 84 per image:

  E = sum_m <(gy (x) I) Abar_m, (I (x) gx) Bbar_m>

Device (2 cores per image, Mc=42 extended classes each, 21 pairs):
  T-stage: per class-pair j, matmul with lhsT = Bbar pair slab [64x, 128(m2,y)]
    and rhs = gx [64,64]: out[(m2,y), x'] — contracting x flips y into the
    output partition, so the result lands directly in the A-side layout.
  P-stage: one matmul per chunk: lhsT = blockdiag(gy,gy) [128,128], rhs = Ay
    [128(m2,y), chunk*64] — applies gy to the packed A side.
  Reduce: fused multiply+reduce (DVE tensor_tensor_reduce / GpSimd
    scalar_tensor_tensor) of the two PSUM chunks into acc columns.
Host sums the per-core [128, n_chunk] partials.
"""

import numpy as np
import ml_dtypes

WEIGHT = 1e-07
SIGMA_RGB = 15.0
SIGMA_XY_EFF = 50.0  # SIGMA_XY * SCALE
IGNORE_LABEL = 255

N_IMG = 4
K_CLS = 21
H_DS = 64
R_COLOR = 4  # 1 + rgb linear Taylor ranks
M_EXT = K_CLS * R_COLOR  # 84
MC = M_EXT // 2  # 42 per core
N_PAIRS = MC // 2  # 21
# (first pair, n pairs, partition row-group); chunks never straddle groups
CHUNKS = [(0, 6, 0), (6, 5, 0), (11, 5, 1), (16, 5, 1)]
NCH = len(CHUNKS)
NB_HALF0 = 11  # pairs 0-10 on partitions 0-63, 11-20 on 64-127

BF16 = ml_dtypes.bfloat16

_CACHE = {}


def _build_program():
    import concourse.bacc as bacc
    import concourse.tile as tile
    from concourse import mybir

    f32 = mybir.dt.float32
    bf16 = mybir.dt.bfloat16

    nc = bacc.Bacc("TRN2", target_bir_lowering=False, debug=False)

    # bxt: [128, 64+11*128]: row half g holds [gxr | pairs 11g..11g+10]
    bxt_d = nc.dram_tensor(
        "bxt", [128, 64 + NB_HALF0 * 128], bf16, kind="ExternalInput"
    )
    # ayg: [gyd | ay pairs 0-20]
    ayg_d = nc.dram_tensor(
        "ayg", [128, 128 + N_PAIRS * 64], bf16, kind="ExternalInput"
    )
    acc_d = nc.dram_tensor("acc_out", [128, NCH], f32, kind="ExternalOutput")

    with tile.TileContext(nc) as tc:
        with (
            tc.tile_pool(name="const", bufs=1) as cpool,
            tc.tile_pool(name="tpsum", bufs=1, space="PSUM") as tpool,
            tc.tile_pool(name="ppsum", bufs=1, space="PSUM") as ppool,
            tc.tile_pool(name="scr", bufs=1) as scrpool,
        ):
            bxt = cpool.tile([128, 64 + NB_HALF0 * 128], bf16, tag="bxt")
            ayg = cpool.tile([128, 128 + N_PAIRS * 64], bf16, tag="ayg")
            acc = cpool.tile([128, NCH], f32, tag="acc")

            nc.sync.dma_start(bxt[:], bxt_d[:])
            nc.scalar.dma_start(ayg[:], ayg_d[:])

            for c, (p0, npair, g) in enumerate(CHUNKS):
                fs = npair * 64
                tp = tpool.tile([128, fs], f32, tag=f"t{c}")
                pp = ppool.tile([128, fs], f32, tag=f"p{c}")
                r0 = 64 * g
                for jj in range(npair):
                    lj = p0 + jj - NB_HALF0 * g
                    nc.tensor.matmul(
                        tp[:, jj * 64 : (jj + 1) * 64],
                        bxt[r0 : r0 + 64, 64 + 128 * lj : 192 + 128 * lj],
                        bxt[r0 : r0 + 64, 0:64],
                        start=True,
                        stop=True,
                        tile_position=(r0, 0),
                        skip_group_check=True,
                    )
                nc.tensor.matmul(
                    pp[:],
                    ayg[:, 0:128],
                    ayg[:, 128 + p0 * 64 : 128 + (p0 + npair) * 64],
                    start=True,
                    stop=True,
                    tile_position=(0, 0),
                )
                tsb = scrpool.tile([128, fs], f32, tag=f"tsb{c}")
                nc.scalar.activation(
                    tsb[:], tp[:], mybir.ActivationFunctionType.Copy
                )
                scr = scrpool.tile([128, fs], f32, tag=f"s{c}")
                nc.vector.scalar_tensor_tensor(
                    out=scr[:],
                    in0=pp[:],
                    scalar=1.0,
                    in1=tsb[:],
                    op0=mybir.AluOpType.mult,
                    op1=mybir.AluOpType.mult,
                    accum_out=acc[:, c : c + 1],
                )

            nc.sync.dma_start(acc_d[:], acc[:])

    nc.compile()
    return nc


def _host_prep(images, segmentations, ROIs, seg_label):
    """Returns the 8 per-core input dicts. Core c -> image c//2, half c%2."""
    imgs = images[:, :, ::2, ::2].astype(np.float64)  # [N,3,64,64]
    segs = (
        segmentations.astype(np.float64)
        .reshape(N_IMG, K_CLS, H_DS, 2, H_DS, 2)
        .mean(axis=(3, 5))
    )  # [N,21,64,64]
    rois = ROIs[:, ::2, ::2].astype(np.float64)
    lbl = seg_label[:, 0, ::2, ::2]
    unlabel = lbl == IGNORE_LABEL

    seg_max = segs.max(axis=1)
    gate = rois - seg_max
    gate = np.where(unlabel, 1.0, gate)
    gate = np.maximum(gate, 0.0)  # [N,64,64]
    seg_r = segs * rois[:, None]  # [N,21,64,64]

    c = imgs / SIGMA_RGB  # [N,3,64,64]
    ec = np.exp(-0.5 * (c**2).sum(axis=1))  # [N,64,64]
    A = seg_r * (gate * ec)[:, None]  # [N,21,64,64]
    B = seg_r * ec[:, None]

    # extended classes m = k*4 + r: r=0 -> (1,1); r=1..3 -> (c_a, c_a)
    phi = np.concatenate([np.ones((N_IMG, 1, H_DS, H_DS)), c], axis=1)  # [N,4,64,64]
    Abar = (A[:, :, None] * phi[:, None]).reshape(N_IMG, M_EXT, H_DS, H_DS)
    Bbar = (B[:, :, None] * phi[:, None]).reshape(N_IMG, M_EXT, H_DS, H_DS)

    idx = np.arange(H_DS, dtype=np.float64)
    g1 = np.exp(-0.5 * ((idx[:, None] - idx[None, :]) / SIGMA_XY_EFF) ** 2)
    g1_16 = g1.astype(BF16)
    gxr = g1_16  # [64,64]
    gyd = np.zeros((128, 128), BF16)
    gyd[0:64, 0:64] = g1_16
    gyd[64:128, 64:128] = g1_16

    in_maps = []
    for core in range(8):
        img_i = core // 2
        half = core % 2
        Al = Abar[img_i, half * MC : (half + 1) * MC]  # [42,64,64]
        Bl = Bbar[img_i, half * MC : (half + 1) * MC]

        # ay: [(m2,y), (pair, x)]; gyd prepended
        A4 = Al.reshape(N_PAIRS, 2, H_DS, H_DS)  # [j, m2, y, x]
        ayf = (
            np.ascontiguousarray(A4.transpose(1, 2, 0, 3))
            .reshape(128, N_PAIRS * 64)
            .astype(BF16)
        )
        ayg = np.concatenate([gyd, ayf], axis=1)

        # bx[x, 128j + m2*64 + y] = Bl[2j+m2, y, x]; gxr prepended
        B4 = Bl.reshape(N_PAIRS, 2, H_DS, H_DS)  # [j, m2, y, x]
        bxf = (
            np.ascontiguousarray(B4.transpose(3, 0, 1, 2))
            .reshape(64, N_PAIRS * 128)
            .astype(BF16)
        )
        bxt = np.zeros((128, 64 + NB_HALF0 * 128), BF16)
        bxt[0:64, 0:64] = gxr
        bxt[64:128, 0:64] = gxr
        bxt[0:64, 64 : 64 + NB_HALF0 * 128] = bxf[:, : NB_HALF0 * 128]
        bxt[64:128, 64 : 64 + (N_PAIRS - NB_HALF0) * 128] = bxf[
            :, NB_HALF0 * 128 :
        ]

        in_maps.append({"bxt": bxt, "ayg": ayg})
    return in_maps


def _get_program():
    if "nc" not in _CACHE:
        _CACHE["nc"] = _build_program()
    return _CACHE["nc"]


def _install_profile_hook():
    """Best-effort registration of the axon NTFF profile hook so that
    trace=True works (used by test harness, not the plain kernel path)."""
    import sys
    import types

    if "antenv.axon_hooks" in sys.modules:
        return
    try:
        from trn_agent_boot.trn_boot import _ntff_profile_via_ctypes

        hook = _ntff_profile_via_ctypes("/opt/axon/libaxon_pjrt.so")
        mod = types.ModuleType("antenv.axon_hooks")
        mod.get_axon_ntff_profile_hook = lambda: hook
        sys.modules["antenv.axon_hooks"] = mod
    except Exception:
        pass


def kernel(images, segmentations, ROIs, seg_label, _trace=False, _tmpdir=None):
    from concourse import bass_utils

    in_maps = _host_prep(images, segmentations, ROIs, seg_label)
    nc = _get_program()
    if _trace:
        _install_profile_hook()
        bass_utils.upload_artifacts = lambda tmpdir: f"local:{tmpdir}"
    res = bass_utils.run_bass_kernel_spmd(
        nc, in_maps, list(range(8)), trace=_trace, tmpdir=_tmpdir
    )
    total = 0.0
    for r in res.results:
        total += r["acc_out"].astype(np.float64).sum()
    loss = np.float32(-WEIGHT / N_IMG * total)
    if _trace:
        return np.array([loss], np.float32), res
    return np.array([loss], np.float32)
